# revision 4
# baseline (speedup 1.0000x reference)
"""DSA sparse attention (context-parallel variant) for Trainium2 via Bass/Tile.

Dense-rewrite algorithm (mathematically identical to the reference):
  w[s,t] = exp(sc[s,t])*ts[s,t] / sum_t' exp(sc)*ts   (softmax->*ts->renorm collapses)
  TS[s,j] = sum_t ts[s,t]*[idx[s,t]==j]  -> dense scatter of score values
  E[s,j]  = TS[s,j]*exp(scale*S[s,j]),  S = Q K^T (dense)
  O       = (E @ V) / rowsum(E)
Everything is computed in transposed layout (kv on partitions); O comes out
natural via E^T-stationary matmuls; rowsum(E) falls out of a ones-column
appended to V.

V3 layout/scheduling notes (over V2):
  - host pre-TRANSPOSES q (pre-scaled) and k to [D, S] layout: the on-chip
    PE transposes (80 matmuls) + DVE evacuation copies disappear and the
    q/k DMAs become perfectly contiguous per partition.
  - host pre-builds the DENSE TS table (bf16, [128, NKV, SGRP] per s-group)
    and the kernel DMAs it instead of running 64 gpsimd local_scatters:
    the Pool engine (47us scatters + 11us drains per rep) drops to zero.
  - S psum tiles are [128, 3, 512] (3 banks) so each ACT exp call covers
    1536 elements instead of 1024, amortizing the ~352-cycle ACT pipeline
    fill; EV accumulators shrink to 2 rotating full-bank tiles (the four
    s-blocks are processed in two half-phases of two blocks each), keeping
    total PSUM usage at exactly 8 banks.
  - phases run g-major: (h0,g0) (h1,g0) (h0,g1) (h1,g1); per phase the S^T
    matmuls are WOVEN with the EV matmuls of the previous phase so the PE
    alternates between ACT-gated S work and dependency-free EV work.
"""

import sys

sys.path.insert(0, "/opt/trn_rl_repo")

import numpy as np

import concourse.bass as bass
import concourse.bacc as bacc
import concourse.mybir as mybir
import concourse.tile as tile
from concourse.vector_clock import ScopedClock

# ---------------------------------------------------------------------------
# Patch: this walrus build encodes at most ONE sync-wait on a CTRL NO_STRUCT
# instruction; TileContext's tail drain carries one wait per live proc.  Split
# the waits across a chain of single-wait drains.
# ---------------------------------------------------------------------------


def _patched_drain_and_barrier(self, tick_clock, wait_clock):
    drain_inst = self.nc.sync.drain()
    wait_clock.add_sem_waits(
        drain_inst.ins, ScopedClock({None: tick_clock.global_clock})
    )
    si = drain_inst.ins.sync_info
    if si is not None and len(si.on_wait) > 1:
        waits = list(si.on_wait)
        drain_inst.ins.sync_info = mybir.SyncInfo(
            on_wait=waits[:1], on_update=list(si.on_update)
        )
        for i in range(1, len(waits)):
            extra = self.nc.sync.drain()
            extra.ins.sync_info = mybir.SyncInfo(on_wait=[waits[i]], on_update=[])
    self.nc.all_engine_barrier()
    assert self.sems is not None
    popped = self.nc._tile_sem_poison_stack.pop()
    assert popped is self._sem_poison
    self.nc.clear_and_free_semaphores(list(self.sems.allocated().values()))
    self.nc.all_engine_barrier()


tile.TileContext._drain_and_barrier = _patched_drain_and_barrier

FP = mybir.dt.float32
BF = mybir.dt.bfloat16


class Cfg:
    def __init__(self, HPC=2, SQ=1024, SKV=4096, D=128, TOPK=64):
        self.HPC = HPC  # heads per core
        self.SQ = SQ
        self.SKV = SKV
        self.D = D
        self.TOPK = TOPK
        self.NKV = SKV // 128  # kv chunks of 128
        self.NSB = SQ // 128  # query blocks of 128
        self.SHALF = 512  # s-group width (s-dim per group)
        self.scale = float(D) ** -0.5


# ---------------------------------------------------------------------------
# Program builder
# ---------------------------------------------------------------------------


def build_program(cfg, nmaxs=None, reps=1):
    nc = bacc.Bacc("TRN2", debug=False)
    HPC, SQ, SKV, D, NKV = cfg.HPC, cfg.SQ, cfg.SKV, cfg.D, cfg.NKV
    NGRP = SQ // cfg.SHALF

    qT = nc.dram_tensor("qT", [HPC, D, SQ], BF, kind="ExternalInput").ap()
    kT = nc.dram_tensor("kT", [HPC, D, SKV], BF, kind="ExternalInput").ap()
    v = nc.dram_tensor("v", [HPC, SKV, D], BF, kind="ExternalInput").ap()
    ts = nc.dram_tensor(
        "ts", [NGRP, 128, NKV, cfg.SHALF], BF, kind="ExternalInput"
    ).ap()
    out = nc.dram_tensor("out", [HPC, SQ, D], FP, kind="ExternalOutput").ap()

    with tile.TileContext(nc) as tc:
        import contextlib

        ctx = contextlib.ExitStack()
        with ctx:
            tst_pool = ctx.enter_context(tc.tile_pool(name="tst", bufs=2))
            ktr_pool = ctx.enter_context(tc.tile_pool(name="ktr", bufs=2))
            et_pool = ctx.enter_context(tc.tile_pool(name="et", bufs=2))
            small_pool = ctx.enter_context(tc.tile_pool(name="small", bufs=4))
            out_pool = ctx.enter_context(tc.tile_pool(name="outp", bufs=4))
            s_psum = ctx.enter_context(tc.tile_pool(name="sps", bufs=2, space="PSUM"))
            ev_psum = ctx.enter_context(tc.tile_pool(name="evp", bufs=2, space="PSUM"))

            def _body(_iv=None):
                _build_body(
                    nc, tc, cfg, qT, kT, v, ts, out,
                    tst_pool, ktr_pool, et_pool, small_pool, out_pool,
                    s_psum, ev_psum,
                )

            if reps == 1:
                _body()
            else:
                with tc.For_i(
                    0, reps, 1,
                    hint_engines=(
                        mybir.EngineType.PE,
                        mybir.EngineType.DVE,
                        mybir.EngineType.Activation,
                        mybir.EngineType.Pool,
                        mybir.EngineType.SP,
                    ),
                ):
                    _body()

    nc.compile()
    return nc


def _build_body(nc, tc, cfg, qT, kT, v, ts, out,
                tst_pool, ktr_pool, et_pool, small_pool, out_pool,
                s_psum, ev_psum):
    HPC, SQ, SKV, D, NKV = cfg.HPC, cfg.SQ, cfg.SKV, cfg.D, cfg.NKV
    SGRP = cfg.SHALF
    NGRP = SQ // SGRP
    NSBG = SGRP // 128  # s-blocks per group (4)
    STG = 16            # kv-chunks per v staging DMA

    # ---------------- input DMAs (contiguous, host-prepped layouts) ---------
    def _load_head(h):
        qtr = ktr_pool.tile([128, SQ], BF, tag="qtr")
        nc.sync.dma_start(qtr[:], qT[h])
        ktr = ktr_pool.tile([128, SKV], BF, tag="ktr")
        nc.sync.dma_start(ktr[:, 0 : SKV // 2], kT[h, :, 0 : SKV // 2])
        nc.sync.dma_start(ktr[:, SKV // 2 : SKV], kT[h, :, SKV // 2 : SKV])
        vaug = ktr_pool.tile([128, NKV, D + 1], BF, tag="vaug")
        vview = v[h].rearrange("(n p) d -> p n d", p=128)
        for gdma in range(NKV // STG):
            nc.sync.dma_start(
                vaug[:, gdma * STG : (gdma + 1) * STG, 0:D],
                vview[:, gdma * STG : (gdma + 1) * STG, :],
            )
        nc.vector.memset(vaug[:, :, D : D + 1], 1.0)
        return qtr, ktr, vaug

    tst0 = tst_pool.tile([128, NKV, SGRP], BF, tag="tst")
    nc.sync.dma_start(tst0[:], ts[0])
    qtr0, ktr0, vaug0 = _load_head(0)
    qtr1, ktr1, vaug1 = _load_head(1)
    tst1 = tst_pool.tile([128, NKV, SGRP], BF, tag="tst")
    nc.sync.dma_start(tst1[:], ts[1])
    qtrs, ktrs, vaugs = [qtr0, qtr1], [ktr0, ktr1], [vaug0, vaug1]
    tsts = [tst0, tst1]

    # ---------------- self-woven compute phases ------------------------------
    phases = [(h, g) for g in range(NGRP) for h in range(HPC)]

    class EvState:
        """EV accumulation for one phase, self-woven into that phase's own
        S-stream with a one-group lag (an et chunk's EV matmuls are emitted
        only after its exp+mul has been emitted).  J-outer order: the four
        s-block accumulators rotate over four half-bank PSUM tiles laid out
        so consecutive matmuls alternate banks (avoids the accumulate RMW
        stall); each block is normalized and stored when its J-loop closes."""

        def __init__(self, h, g, et):
            self.h, self.g, self.et = h, g, et
            self.pos = 0  # number of (J, b) steps emitted; J = pos//4, b = pos%4
            self.ops = [None] * NSBG

        def emit_upto(self, j_ready):
            """Emit EV matmuls for all chunks J < j_ready."""
            vaug = vaugs[self.h]
            while self.pos < 4 * j_ready:
                J, b = divmod(self.pos, 4)
                if J == 0:
                    self.ops[b] = ev_psum.tile(
                        [128, 256], FP, tag="evacc", name="evacc"
                    )
                nc.tensor.matmul(
                    self.ops[b][:, 0 : D + 1],
                    self.et[:, J, b * 128 : (b + 1) * 128],
                    vaug[:, J, :],
                    start=(J == 0), stop=(J == NKV - 1),
                )
                if J == NKV - 1:
                    sb = self.g * NSBG + b
                    recip = small_pool.tile([128, 1], FP, tag="recip")
                    nc.vector.reciprocal(recip[:], self.ops[b][:, D : D + 1])
                    ot = out_pool.tile([128, D], FP, tag="ot")
                    nc.vector.tensor_scalar_mul(ot[:], self.ops[b][:, 0:D], recip[:])
                    nc.sync.dma_start(
                        out[self.h, sb * 128 : (sb + 1) * 128, :], ot[:]
                    )
                self.pos += 1

        def finish(self):
            self.emit_upto(NKV)

    TRIP = 3
    groups = [TRIP] * (NKV // TRIP) + ([NKV % TRIP] if NKV % TRIP else [])

    prev = None  # EvState carrying the previous phase's small EV remainder
    for pi, (h, g) in enumerate(phases):
        qtr, ktr, tst = qtrs[h], ktrs[h], tsts[g]
        et = et_pool.tile([128, NKV, SGRP], BF, tag="et")
        sl = slice(g * SGRP, (g + 1) * SGRP)
        ev = EvState(h, g, et)
        J = 0
        for w in groups:
            sp = s_psum.tile([128, TRIP, SGRP], FP, tag="sps")
            for t in range(w):
                nc.tensor.matmul(
                    sp[:, t, :],
                    ktr[:, (J + t) * 128 : (J + t + 1) * 128],
                    qtr[:, sl],
                    start=True, stop=True,
                )
            # weave: drain the previous phase's EV remainder, then this
            # phase's own EV matmuls for every chunk already exp+mul'd
            if prev is not None:
                prev.finish()
                prev = None
            ev.emit_upto(J)
            nc.scalar.activation(
                et[:, J : J + w, :], sp[:, 0:w, :],
                mybir.ActivationFunctionType.Exp,
            )
            nc.vector.tensor_mul(
                et[:, J : J + w, :], et[:, J : J + w, :], tst[:, J : J + w, :]
            )
            J += w
        prev = ev

    # tail: the last phase's final EV group runs unwoven (small)
    if prev is not None:
        prev.finish()


# ---------------------------------------------------------------------------
# Entry point: full unsharded inputs -> full output.
# Sharding: head-parallel, 2 heads per NeuronCore across 8 cores; the
# topk index/score tensors are shared by all cores.
# ---------------------------------------------------------------------------

_CACHE = {}


def make_in_maps(q, k, v, topk_indices, topk_scores, cfg):
    """Host-side prep: bf16 conversion, q pre-scaling + transpose, k
    transpose, dense TS table build.  Returns (in_maps, nmaxs)."""
    import ml_dtypes

    bf16 = ml_dtypes.bfloat16
    SQ, SKV, NKV, SGRP = cfg.SQ, cfg.SKV, cfg.NKV, cfg.SHALF
    NGRP = SQ // SGRP

    # dense TS[j, s] = sum of topk_scores over duplicate (s, j) selections
    idx = np.asarray(topk_indices)[0].astype(np.int64)          # [SQ, TOPK]
    sc = np.asarray(topk_scores, dtype=np.float32)[0]           # [SQ, TOPK]
    tsd = np.zeros((SKV, SQ), dtype=np.float32)                 # [j, s]
    s_arr = np.repeat(np.arange(SQ, dtype=np.int64), cfg.TOPK)
    np.add.at(tsd, (idx.reshape(-1), s_arr), sc.reshape(-1))
    # per group: [128, NKV, SGRP] with ts[p, J, s] = tsd[J*128 + p, g*SGRP + s]
    tsd = tsd.reshape(NKV, 128, NGRP, SGRP).transpose(2, 1, 0, 3)  # [g,p,J,s]
    ts_bf = np.ascontiguousarray(tsd.astype(bf16))

    qs = (np.asarray(q, dtype=np.float32) * (float(cfg.D) ** -0.5)).astype(bf16)
    kb = np.asarray(k, dtype=np.float32).astype(bf16)
    vb = np.asarray(v, dtype=np.float32).astype(bf16)
    qsT = np.ascontiguousarray(qs[0].transpose(0, 2, 1))  # [H, D, SQ]
    kbT = np.ascontiguousarray(kb[0].transpose(0, 2, 1))  # [H, D, SKV]

    in_maps = []
    for i in range(8):
        m = {
            "qT": np.ascontiguousarray(qsT[2 * i : 2 * i + 2]),
            "kT": np.ascontiguousarray(kbT[2 * i : 2 * i + 2]),
            "v": np.ascontiguousarray(vb[0, 2 * i : 2 * i + 2]),
            "ts": ts_bf,
        }
        in_maps.append(m)
    return in_maps, ()


def kernel(q, k, v, topk_indices, topk_scores):
    q = np.asarray(q, dtype=np.float32)
    B, H, SQ, D = q.shape
    SKV = np.asarray(k).shape[2]
    TOPK = np.asarray(topk_indices).shape[-1]
    assert B == 1 and H == 16 and SQ == 1024 and SKV == 4096 and D == 128

    cfg = Cfg(HPC=H // 8, SQ=SQ, SKV=SKV, D=D, TOPK=TOPK)
    in_maps, nmaxs = make_in_maps(q, k, v, topk_indices, topk_scores, cfg)

    nc = _CACHE.get("v3")
    if nc is None:
        nc = build_program(cfg, list(nmaxs), reps=1)
        _CACHE["v3"] = nc

    from concourse.bass_utils import run_bass_kernel_spmd

    res = run_bass_kernel_spmd(nc, in_maps, list(range(8)))
    out = np.stack([res.results[i]["out"] for i in range(8)])
    return out.reshape(1, H, SQ, D).astype(np.float32)
